# revision 13
# baseline (speedup 1.0000x reference)
"""Bolmo attention (GQA + QK-RMSNorm + RoPE + causal attention + out-proj)
as an 8-way tensor-parallel Bass kernel for one TRN2 chip.

Sharding: head-parallel. Core c owns Q heads [4c, 4c+4) (wq columns
[256c, 256c+256)), KV head c (wk/wv columns [64c, 64c+64)), and wo rows
[256c, 256c+256). hidden_states is replicated. Each core computes a partial
output (its heads' contribution through wo); the host sums the 8 partials.

The QK RMSNorm is over the FULL projected dim (2048 for q, 512 for k), so the
per-core sum-of-squares partials are combined with one tiny on-device
AllReduce ([2, 2048] fp32).

On-chip dataflow is feature-major: Q^T [dq, tok], K^T [dk, tok] come straight
out of the projection matmuls, attention scores are computed transposed
(S^T = K @ Q^T, k on partitions) so softmax+AV need no transposes, and the
softmax row-sums come free from an all-ones row appended to V. Causality is
structural (upper-triangular tiles skipped, diagonal tiles masked with
affine_select); the host verifies the provided attention_mask really is the
standard causal mask and falls back to a numpy path if not.

SBUF layout notes: matmul operands need partition base in {0,32,64} and
lhsT/rhs bases equal, so all four Q heads live base-0 in one [64, 4T] tile;
K^T and V^T share one [128, T] tile (K rows 0:64, V rows 64:128, with a
second identity staged at base 64 for the V transposes).
"""

import os
import sys

import numpy as np

for _p in ("/opt/trn_rl_repo", "/root/.axon_site/_ro/trn_rl_repo"):
    if os.path.isdir(_p) and _p not in sys.path:
        sys.path.insert(0, _p)

from concourse import bacc, masks, mybir, tile  # noqa: E402
from concourse.bass_utils import run_bass_kernel_spmd  # noqa: E402

B, S, H = 2, 1024, 2048
NH, NKV, HD = 32, 8, 64
T = B * S
NCORES = 8
DQ = (NH // NCORES) * HD     # 256 q dims per core
DK = (NKV // NCORES) * HD    # 64 kv dims per core
EPS = 1e-6
SCALE = HD ** -0.5

F32 = mybir.dt.float32
F32R = mybir.dt.float32r
AF = mybir.ActivationFunctionType
ALU = mybir.AluOpType

NT = T // 128       # 16 token tiles
NHT = H // 128      # 16 hidden tiles
NCH = T // 512      # 4 token chunks (rope)
NHC = T // 256      # 8 half-chunks (projections)
SKT = S // 128      # 8 key tiles per batch
SQC = S // 256      # 4 query chunks per batch


def build(debug=False):
    nc = bacc.Bacc("TRN2", target_bir_lowering=False, debug=False,
                   num_devices=NCORES)

    hs = nc.dram_tensor("hs", [T, H], F32, kind="ExternalInput").ap()
    wq = nc.dram_tensor("wq", [H, DQ], F32, kind="ExternalInput").ap()
    wk = nc.dram_tensor("wk", [H, DK], F32, kind="ExternalInput").ap()
    wv = nc.dram_tensor("wv", [H, DK], F32, kind="ExternalInput").ap()
    wo = nc.dram_tensor("wo", [DQ, H], F32, kind="ExternalInput").ap()
    qnw = nc.dram_tensor("qnw", [DQ], F32, kind="ExternalInput").ap()
    knw = nc.dram_tensor("knw", [DK], F32, kind="ExternalInput").ap()
    cos = nc.dram_tensor("cos", [T, HD], F32, kind="ExternalInput").ap()
    sin = nc.dram_tensor("sin", [T, HD], F32, kind="ExternalInput").ap()
    out = nc.dram_tensor("out", [T, H], F32, kind="ExternalOutput").ap()
    if debug:
        dbg_q = nc.dram_tensor("dbg_q", [DQ, T], F32, kind="ExternalOutput").ap()
        dbg_k = nc.dram_tensor("dbg_k", [DK, T], F32, kind="ExternalOutput").ap()
        dbg_r = nc.dram_tensor("dbg_r", [2, T], F32, kind="ExternalOutput").ap()
        dbg_ot = nc.dram_tensor("dbg_ot", [DQ, T], F32, kind="ExternalOutput").ap()

    with tile.TileContext(nc) as tc:
        with (
            tc.tile_pool(name="wpool", bufs=1) as wpool,
            tc.tile_pool(name="persist", bufs=1) as persist,
            tc.tile_pool(name="dram", bufs=1, space="DRAM") as dram,
        ):
            # ---------------- constants & weights ----------------
            # (memset can't write f32r; build constants in f32, cast via ACT)
            idf = wpool.tile([128, 128], F32, tag="idf")
            masks.make_identity(nc, idf[:])
            onesf = wpool.tile([128, 1], F32, tag="onesf")
            nc.gpsimd.memset(onesf[:], 1.0)
            ident = wpool.tile([128, 128], F32R, tag="ident")
            nc.scalar.copy(ident[:], idf[:])
            # identity staged at partition base 64 (for V^T transposes)
            ident2 = wpool.tile([128, 64], F32R, tag="ident2")
            nc.scalar.copy(ident2[0:64, :], idf[0:64, 0:64])
            nc.scalar.copy(ident2[64:128, :], idf[0:64, 0:64])
            ones128 = wpool.tile([128, 1], F32R, tag="ones128")
            nc.scalar.copy(ones128[:], onesf[:])
            eps1 = wpool.tile([1, 1], F32, tag="eps1")
            nc.gpsimd.memset(eps1[:], EPS)

            wq_sb = wpool.tile([128, NHT * DQ], F32R, tag="wq_sb")
            wk_sb = wpool.tile([128, NHT * DK], F32R, tag="wk_sb")
            wv_sb = wpool.tile([128, NHT * DK], F32R, tag="wv_sb")
            for hh in range(NHT):
                nc.sync.dma_start(wq_sb[:, hh * DQ:(hh + 1) * DQ],
                                  wq[hh * 128:(hh + 1) * 128, :].bitcast(F32R))
                nc.sync.dma_start(wk_sb[:, hh * DK:(hh + 1) * DK],
                                  wk[hh * 128:(hh + 1) * 128, :].bitcast(F32R))
                nc.sync.dma_start(wv_sb[:, hh * DK:(hh + 1) * DK],
                                  wv[hh * 128:(hh + 1) * 128, :].bitcast(F32R))
            wo_sb = wpool.tile([128, 2 * H], F32R, tag="wo_sb")
            for m in range(2):
                nc.sync.dma_start(wo_sb[:, m * H:(m + 1) * H],
                                  wo[m * 128:(m + 1) * 128, :].bitcast(F32R))
            qnw_sb = wpool.tile([128, 2], F32, tag="qnw_sb")
            nc.sync.dma_start(qnw_sb[:], qnw.rearrange("(m p) -> p m", p=128))
            knw_sb = wpool.tile([64, 1], F32, tag="knw_sb")
            nc.sync.dma_start(knw_sb[:], knw.rearrange("(m p) -> p m", p=64))

            # persistent activations
            qa = persist.tile([64, 4 * T], F32R, tag="qa")  # Q^T, head-major
            kv = persist.tile([128, T], F32R, tag="kv")     # K^T 0:64, V^T 64:128
            oT = [persist.tile([128, T], F32R, tag=f"oT{m}", name=f"oT{m}")
                  for m in range(2)]
            rstd_q = persist.tile([1, T], F32, tag="rstd_q")
            rstd_k = persist.tile([1, T], F32, tag="rstd_k")

            # ---------------- cos/sin -> feature-major ----------
            # cos2 [64,T] = cos^T.  sin2 [64,T] row-shifted, sign-baked:
            # rows 0:32 = +sin^T[32:64], rows 32:64 = -sin^T[0:32].
            cos2 = wpool.tile([64, T], F32, tag="cos2")
            sin2 = wpool.tile([64, T], F32, tag="sin2")
            with tc.tile_pool(name="cs_pool", bufs=2) as cspool, \
                 tc.tile_pool(name="cs_psum", bufs=2, space="PSUM") as cspp:
                for src_, dst in ((cos, cos2), (sin, sin2)):
                    cs_in = cspool.tile([128, NT * HD], F32R, tag="cs_in")
                    for tt in range(NT):
                        nc.sync.dma_start(
                            cs_in[:, tt * HD:(tt + 1) * HD],
                            src_[tt * 128:(tt + 1) * 128, :].bitcast(F32R))
                    for c4 in range(NCH):
                        tp = cspp.tile([64, 512], F32R, tag="cs_tp")
                        for j in range(4):
                            tt = c4 * 4 + j
                            nc.tensor.transpose(
                                tp[:, j * 128:(j + 1) * 128],
                                cs_in[:, tt * HD:(tt + 1) * HD],
                                ident[:])
                        nc.scalar.copy(dst[:, c4 * 512:(c4 + 1) * 512],
                                       tp[:].bitcast(F32))
                stmp = cspool.tile([32, T], F32, tag="stmp")
                nc.vector.tensor_copy(stmp[:], sin2[0:32, :])
                nc.vector.tensor_copy(sin2[0:32, :], sin2[32:64, :])
                nc.vector.tensor_copy(sin2[32:64, :], stmp[:])
                nc.vector.tensor_scalar_mul(sin2[32:64, :], sin2[32:64, :], -1.0)

            cc_in = dram.tile([2, T], F32)
            cc_out = dram.tile([2, T], F32)

            # ---------------- phase 1: hs^T + projections + ssq --------
            with tc.tile_pool(name="h_pool", bufs=1) as hpool, \
                 tc.tile_pool(name="w1_pool", bufs=2) as w1, \
                 tc.tile_pool(name="p1_psum", bufs=1, space="PSUM") as pp1, \
                 tc.tile_pool(name="p1t_psum", bufs=2, space="PSUM") as pp1t:
                for c4 in range(NCH):
                    cols = slice(c4 * 512, (c4 + 1) * 512)
                    hsT = hpool.tile([128, NHT * 512], F32R, tag="hsT")
                    hsT3 = hsT[:].rearrange("p (h f) -> p h f", f=512)
                    for j in range(4):
                        tt = c4 * 4 + j
                        hs_in = w1.tile([128, H], F32R, tag="hs_in")
                        nc.sync.dma_start(
                            hs_in[:],
                            hs[tt * 128:(tt + 1) * 128, :].bitcast(F32R))
                        for hq in range(4):
                            tp = pp1t.tile([128, 512], F32R, tag="tp")
                            for hi in range(4):
                                hh = hq * 4 + hi
                                nc.tensor.transpose(
                                    tp[:, hi * 128:(hi + 1) * 128],
                                    hs_in[:, hh * 128:(hh + 1) * 128],
                                    ident[:])
                            dst = hsT3[:, hq * 4:(hq + 1) * 4,
                                       j * 128:(j + 1) * 128]
                            tps = tp[:].rearrange("p (a b) -> p a b", a=4)
                            if hq % 2 == 0:
                                nc.scalar.copy(dst, tps)
                            else:
                                nc.vector.tensor_copy(dst, tps)
                    # projections for this chunk
                    ssq_ps = pp1.tile([1, 512], F32, tag="ssq_ps")
                    ssq_pk = pp1.tile([1, 512], F32, tag="ssq_pk")
                    for m in range(2):
                        pq = pp1.tile([128, 512], F32, tag=f"pq{m}",
                                      name=f"pq{m}")
                        for hh in range(NHT):
                            nc.tensor.matmul(
                                pq[:],
                                wq_sb[:, hh * DQ + m * 128:
                                      hh * DQ + (m + 1) * 128],
                                hsT[:, hh * 512:(hh + 1) * 512],
                                start=(hh == 0), stop=(hh == NHT - 1))
                        qsq = w1.tile([128, 512], F32R, tag="qsq")
                        nc.scalar.square(qsq[:], pq[:])
                        nc.tensor.matmul(ssq_ps[0:1, :], ones128[:], qsq[:],
                                         start=(m == 0), stop=(m == 1))
                        # pre-norm q -> qa blocks (head-major, base 0)
                        he, ho = 2 * m, 2 * m + 1
                        nc.scalar.activation(
                            qa[:, he * T + c4 * 512: he * T + (c4 + 1) * 512],
                            pq[0:64, :], AF.Copy, scale=qnw_sb[0:64, m:m + 1])
                        nc.scalar.activation(
                            qa[:, ho * T + c4 * 512: ho * T + (c4 + 1) * 512],
                            pq[64:128, :], AF.Copy,
                            scale=qnw_sb[64:128, m:m + 1])
                    pk = pp1.tile([64, 512], F32, tag="pk")
                    pv = pp1.tile([64, 512], F32, tag="pv")
                    for hh in range(NHT):
                        nc.tensor.matmul(
                            pk[:], wk_sb[:, hh * DK:(hh + 1) * DK],
                            hsT[:, hh * 512:(hh + 1) * 512],
                            start=(hh == 0), stop=(hh == NHT - 1))
                    for hh in range(NHT):
                        nc.tensor.matmul(
                            pv[:], wv_sb[:, hh * DK:(hh + 1) * DK],
                            hsT[:, hh * 512:(hh + 1) * 512],
                            start=(hh == 0), stop=(hh == NHT - 1))
                    ksq = w1.tile([64, 512], F32R, tag="qsq", name="ksq")
                    nc.scalar.square(ksq[:], pk[:])
                    nc.tensor.matmul(ssq_pk[0:1, :], ones128[0:64, :],
                                     ksq[:], start=True, stop=True)
                    nc.scalar.activation(kv[0:64, cols], pk[:],
                                         AF.Copy, scale=knw_sb[:, 0:1])
                    nc.scalar.copy(kv[64:128, cols], pv[:])
                    st_q = w1.tile([1, 512], F32, tag="st_q")
                    st_k = w1.tile([1, 512], F32, tag="st_k")
                    nc.vector.tensor_copy(st_q[:], ssq_ps[0:1, :])
                    nc.vector.tensor_copy(st_k[:], ssq_pk[0:1, :])
                    nc.sync.dma_start(cc_in[0:1, cols], st_q[:])
                    nc.sync.dma_start(cc_in[1:2, cols], st_k[:])

            # ---------------- ssq AllReduce + rms factors --------------
            nc.gpsimd.collective_compute(
                "AllReduce", ALU.add,
                ins=[cc_in.opt()], outs=[cc_out.opt()],
                replica_groups=[list(range(NCORES))],
            )
            nc.sync.dma_start(rstd_q[:], cc_out[0:1, :])
            nc.sync.dma_start(rstd_k[:], cc_out[1:2, :])
            nc.scalar.activation(rstd_q[:], rstd_q[:], AF.Sqrt,
                                 bias=eps1[:], scale=1.0 / (NH * HD))
            nc.scalar.activation(rstd_k[:], rstd_k[:], AF.Sqrt,
                                 bias=eps1[:], scale=1.0 / (NKV * HD))
            nc.vector.reciprocal(rstd_q[:], rstd_q[:])
            nc.vector.reciprocal(rstd_k[:], rstd_k[:])
            if debug:
                nc.sync.dma_start(dbg_r[0:1, :], rstd_q[:])
                nc.sync.dma_start(dbg_r[1:2, :], rstd_k[:])

            # ---------------- phase 2: norm + rope (per head) ----------
            with tc.tile_pool(name="w2_pool", bufs=2) as w2:
                for c4 in range(NCH):
                    cols = slice(c4 * 512, (c4 + 1) * 512)
                    rqb = w2.tile([64, 512], F32, tag="rqb")
                    nc.gpsimd.partition_broadcast(rqb[:], rstd_q[:, cols])
                    rkb = w2.tile([64, 512], F32, tag="rkb")
                    nc.gpsimd.partition_broadcast(rkb[:], rstd_k[:, cols])
                    blocks = [(qa[:, h * T + c4 * 512: h * T + (c4 + 1) * 512],
                               rqb) for h in range(4)]
                    blocks.append((kv[0:64, cols], rkb))
                    for i, (blk, rb_) in enumerate(blocks):
                        t1 = w2.tile([64, 512], F32, tag="t1",
                                     name=f"t1_{c4}_{i}")
                        t2 = w2.tile([64, 512], F32, tag="t2",
                                     name=f"t2_{c4}_{i}")
                        t3 = w2.tile([64, 512], F32, tag="t3",
                                     name=f"t3_{c4}_{i}")
                        nc.vector.tensor_mul(t1[:], blk.bitcast(F32), rb_[:])
                        nc.vector.tensor_mul(t2[:], t1[:], cos2[:, cols])
                        nc.vector.tensor_mul(t3[0:32, :], t1[32:64, :],
                                             sin2[32:64, cols])
                        nc.vector.tensor_mul(t3[32:64, :], t1[0:32, :],
                                             sin2[0:32, cols])
                        nc.vector.tensor_add(blk, t2[:], t3[:])
            if debug:
                for h in range(4):
                    nc.sync.dma_start(dbg_q[h * 64:(h + 1) * 64, :],
                                      qa[:, h * T:(h + 1) * T].bitcast(F32))
                nc.sync.dma_start(dbg_k[:], kv[0:64, :].bitcast(F32))

            # ---------------- phase 3: attention (512-wide q chunks) ----
            with tc.tile_pool(name="a_pool", bufs=2) as apool, \
                 tc.tile_pool(name="pt_pool", bufs=4) as ptpool, \
                 tc.tile_pool(name="sm_pool", bufs=3) as smpool, \
                 tc.tile_pool(name="pa_psum", bufs=2, space="PSUM") as ppa, \
                 tc.tile_pool(name="pav_psum", bufs=2, space="PSUM") as ppav:
                for b in range(B):
                    boff = b * S
                    # V^T -> token-major [128, 65] blocks (ones row appended)
                    vta = apool.tile([128, SKT * 65], F32R, tag="vta")
                    vtp = ppav.tile([128, 512], F32R, tag="vtp")
                    for ki in range(SKT):
                        nc.tensor.transpose(
                            vtp[:, ki * 64: ki * 64 + 64],
                            kv[64:128, boff + ki * 128: boff + (ki + 1) * 128],
                            ident2[64:128, :])
                    for ki in range(SKT):
                        nc.scalar.copy(vta[:, ki * 65: ki * 65 + 64],
                                       vtp[:, ki * 64:(ki + 1) * 64])
                        nc.scalar.copy(vta[:, ki * 65 + 64: ki * 65 + 65],
                                       onesf[:])
                    for h in range(4):
                        m, prow = h // 2, (h % 2) * 64
                        for qj in range(2):
                            qc0 = boff + qj * 512
                            nkt = 4 * qj + 4
                            ovp = ppav.tile([65, 512], F32, tag="ovp")
                            for ki in range(nkt):
                                stp = ppa.tile([128, 512], F32, tag="stp")
                                nc.tensor.matmul(
                                    stp[:],
                                    kv[0:64, boff + ki * 128:
                                       boff + (ki + 1) * 128],
                                    qa[:, h * T + qc0: h * T + qc0 + 512],
                                    start=True, stop=True)
                                pt = ptpool.tile([128, 512], F32R, tag="pt")
                                nc.scalar.activation(pt[:], stp[:], AF.Exp,
                                                     scale=SCALE)
                                if ki >= 4 * qj:
                                    nc.gpsimd.affine_select(
                                        pt[:], pt[:],
                                        pattern=[[1, 512]],
                                        base=qj * 512 - ki * 128,
                                        channel_multiplier=-1,
                                        compare_op=ALU.is_ge,
                                        fill=0.0)
                                nc.tensor.matmul(
                                    ovp[:],
                                    vta[:, ki * 65:(ki + 1) * 65],
                                    pt[:],
                                    start=(ki == 0), stop=(ki == nkt - 1))
                            recip = smpool.tile([1, 512], F32, tag="recip")
                            nc.vector.reciprocal(recip[:], ovp[64:65, :])
                            rb = smpool.tile([64, 512], F32, tag="rb")
                            nc.gpsimd.partition_broadcast(rb[:], recip[:])
                            nc.vector.tensor_mul(
                                oT[m][prow:prow + 64, qc0:qc0 + 512],
                                ovp[0:64, :], rb[:])
            if debug:
                for m in range(2):
                    nc.sync.dma_start(dbg_ot[m * 128:(m + 1) * 128, :],
                                      oT[m][:].bitcast(F32))

            # ---------------- phase 4: output projection ----------------
            with tc.tile_pool(name="w4_pool", bufs=4) as w4, \
                 tc.tile_pool(name="po_psum", bufs=4, space="PSUM") as ppo:
                for tt in range(NT):
                    for nj in range(4):
                        po = ppo.tile([128, 512], F32, tag="po")
                        for m in range(2):
                            nc.tensor.matmul(
                                po[:],
                                oT[m][:, tt * 128:(tt + 1) * 128],
                                wo_sb[:, m * H + nj * 512:
                                      m * H + (nj + 1) * 512],
                                start=(m == 0), stop=(m == 1))
                        outc = w4.tile([128, 512], F32, tag="outc")
                        if nj % 2 == 0:
                            nc.scalar.copy(outc[:], po[:])
                        else:
                            nc.vector.tensor_copy(outc[:], po[:])
                        nc.sync.dma_start(
                            out[tt * 128:(tt + 1) * 128,
                                nj * 512:(nj + 1) * 512],
                            outc[:])
    nc.compile()
    return nc


_CACHED = {}


def _get_nc(debug=False):
    if debug not in _CACHED:
        _CACHED[debug] = build(debug)
    return _CACHED[debug]


def _is_causal_mask(mask):
    m = np.asarray(mask)
    if m.shape != (B, 1, S, S):
        return False
    tri = np.tril(np.ones((S, S), dtype=bool))
    for b in range(B):
        mb = m[b, 0]
        if not np.all(mb[tri] == 0.0):
            return False
        if not np.all(mb[~tri] <= -1e8):
            return False
    return True


def _numpy_fallback(hidden_states, cos, sin, attention_mask, wq, wk, wv, wo,
                    q_norm_w, k_norm_w):
    hs = np.asarray(hidden_states, np.float64)
    b, s, _ = hs.shape
    g = NH // NKV

    def rms(x, w):
        var = np.mean(x * x, axis=-1, keepdims=True)
        return w * (x / np.sqrt(var + EPS))

    def rot(x):
        x1, x2 = np.split(x, 2, axis=-1)
        return np.concatenate((-x2, x1), axis=-1)

    q = rms(hs @ np.asarray(wq, np.float64), np.asarray(q_norm_w, np.float64))
    k = rms(hs @ np.asarray(wk, np.float64), np.asarray(k_norm_w, np.float64))
    v = hs @ np.asarray(wv, np.float64)
    q = q.reshape(b, s, NH, HD).transpose(0, 2, 1, 3)
    k = k.reshape(b, s, NKV, HD).transpose(0, 2, 1, 3)
    v = v.reshape(b, s, NKV, HD).transpose(0, 2, 1, 3)
    c = np.asarray(cos, np.float64)[:, None]
    sn = np.asarray(sin, np.float64)[:, None]
    q = q * c + rot(q) * sn
    k = k * c + rot(k) * sn
    k = np.repeat(k, g, axis=1)
    v = np.repeat(v, g, axis=1)
    sc = np.einsum('bhqd,bhkd->bhqk', q, k) * SCALE + np.asarray(
        attention_mask, np.float64)
    sc = sc - sc.max(axis=-1, keepdims=True)
    e = np.exp(sc)
    attn = e / e.sum(axis=-1, keepdims=True)
    o = np.einsum('bhqk,bhkd->bhqd', attn, v)
    o = o.transpose(0, 2, 1, 3).reshape(b, s, NH * HD)
    return (o @ np.asarray(wo, np.float64)).astype(np.float32)


def make_in_maps(hidden_states, cos, sin, wq, wk, wv, wo, q_norm_w, k_norm_w):
    hsf = np.ascontiguousarray(
        np.asarray(hidden_states, np.float32).reshape(T, H))
    cosf = np.ascontiguousarray(np.asarray(cos, np.float32).reshape(T, HD))
    sinf = np.ascontiguousarray(np.asarray(sin, np.float32).reshape(T, HD))
    in_maps = []
    for c in range(NCORES):
        qs = slice(c * DQ, (c + 1) * DQ)
        ks = slice(c * DK, (c + 1) * DK)
        in_maps.append({
            "hs": hsf,
            "wq": np.ascontiguousarray(np.asarray(wq, np.float32)[:, qs]),
            "wk": np.ascontiguousarray(np.asarray(wk, np.float32)[:, ks]),
            "wv": np.ascontiguousarray(np.asarray(wv, np.float32)[:, ks]),
            "wo": np.ascontiguousarray(np.asarray(wo, np.float32)[qs, :]),
            "qnw": np.ascontiguousarray(np.asarray(q_norm_w, np.float32)[qs]),
            "knw": np.ascontiguousarray(np.asarray(k_norm_w, np.float32)[ks]),
            "cos": cosf,
            "sin": sinf,
        })
    return in_maps


def run(inputs, debug=False, trace=False):
    nc = _get_nc(debug)
    in_maps = make_in_maps(
        inputs["hidden_states"], inputs["cos"], inputs["sin"],
        inputs["wq"], inputs["wk"], inputs["wv"], inputs["wo"],
        inputs["q_norm_w"], inputs["k_norm_w"])
    return run_bass_kernel_spmd(nc, in_maps, list(range(NCORES)), trace=trace)


def kernel(hidden_states, cos, sin, attention_mask, wq, wk, wv, wo,
           q_norm_w, k_norm_w):
    if not _is_causal_mask(attention_mask):
        return _numpy_fallback(hidden_states, cos, sin, attention_mask,
                               wq, wk, wv, wo, q_norm_w, k_norm_w)
    res = run({"hidden_states": hidden_states, "cos": cos, "sin": sin,
               "wq": wq, "wk": wk, "wv": wv, "wo": wo,
               "q_norm_w": q_norm_w, "k_norm_w": k_norm_w})
    total = np.zeros((T, H), np.float64)
    for c in range(NCORES):
        total += res.results[c]["out"].astype(np.float64)
    return total.reshape(B, S, H).astype(np.float32)
